# revision 113
# baseline (speedup 1.0000x reference)
"""ANI AEV computer on 8 TRN2 NeuronCores (Bass/Tile), data-parallel over molecules.

v13 (~47.2us, vs 75.7us baseline). Key structure:
- merged-tile per-atom phase: both 96-row tiles (2 molecules each) processed
  in one instruction stream via a free-axis t dimension (engine cost depends
  only on free size, so this halves per-op overhead vs two passes)
- selection key w = relu(RCA^2 - d2)*selfmask: out-of-cutoff and empty
  slots decode to d = RCA where fc = 0 kills their weight naturally (no
  explicit valid/vkill masking ops needed)
- neighbor gather via select-mask == top-k key value (no max_index), with
  x,y packed as exact 11-bit integers in one f32 word (single mask mult +
  reduce gathers both; unpacked with the f32 round-to-int trick) and z in f16
- f16 2x-mode DVE ops wherever precision allows; f32 kept for tb (cos
  half-angle, condition number 64) and the S/SQ2/LW channel matmuls
- z-major terms layout: each terms op fires right after its per-z Exp;
  bucket matmuls pack 3 groups per PSUM bank (bases 0/32/64) so only 6
  PSUM->SBUF copies + 2 DMAs drain the angular output
- per-chunk clip so the lnc/chp/tb/Ln chain starts before the lwb adds
- emission ordered so the Pool fc_a polynomial chain runs concurrently with
  the DVE gather (az issued pre-gather; LW tail lands right after it), and
  the angular output drains in three DMA pieces issued as banks complete
- slot channels split hi/lo into f16 pairs so the pair matmuls run at
  1 cyc/row while accumulating exact f32 values in PSUM
Shards (32,48) species / (32,48,3) coords over 8 cores (4 molecules each),
returns (32,48,384) float32.
"""
import math
import numpy as np

RCR, RCA = 5.2, 3.5
ETA_R = 16.0
SHFA = np.linspace(0.9, 3.5, 5)[:-1].astype(np.float64)
SHFZ = (np.linspace(0.0, math.pi, 9)[:-1] + math.pi / 16.0).astype(np.float64)
LN2H = 0.5 * math.log(2.0)
K = 20                              # 4 species x 5 slots
IA, IB = np.triu_indices(K, 1)
NPAIR = len(IA)                     # 190
CW = 95                             # pairs per chunk
EPS2 = 0.01
CSEL = RCA * RCA                    # selection key offset: w<=0 outside cutoff
QS = 2047.0 / 10.0                  # 11-bit coordinate quantization

# fc = 0.5 - 0.5*sin(pi*(d/rc-0.5)); odd poly deg 7 for sin(pi*z), z in [-.5,.5]
_z = np.linspace(-0.5, 0.5, 20001)
SINC, *_ = np.linalg.lstsq(np.stack([_z, _z**3, _z**5, _z**7], 1),
                           np.sin(np.pi * _z), rcond=None)

# ---- const pack layout (f32 cols, [128, C_W]) ----
_C_JROW = 0                          # [128, 48] iota j
_C_SHROW = 48                        # [128, 768] radial shifts f-major
_C_SC = 816                          # [128, 24] scalar const columns
_C_SHFA = 840                        # [128, 160] shfa row (m*40+u -> SHFA[m])
_C_BW = 1000                         # [128, 160] bw_m/2 row (m*40+u)
_C_BZ = 1160                         # [128, 8] f1 exp bias 64*ln(cos(shfz/2))
_C_B975 = 1168                       # 0.975
_C_B025 = 1169                       # 0.025
_C_EXPD16 = 1170                     # [128, 95] f32 = [190] f16, at bases 0/32/64
_C_EXPS = 1265                       # [128, 190] f32 exps, at bases 0/32/64/96
_C_BUCK = 1455                       # [128, 32] f32 = [64] f16 bucket onehot (pad 32)
_C_IDT16 = 1487                      # [128, 48] f32 = [96] f16 identity
_C_IDT32 = 1535                      # [128, 96] f32 identity
_C_EXPS16 = 1631                     # [128, 95] f32 = [190] f16 exps @0/32/64
_C_W = 1726

# scalar const column values
#  0-3: sin poly c7,c5,c3,c1;  4: 1/RCR; 5: -0.5; 6: 0.125; 7: -0.125
#  8: 1/RCA; 9: 0.5; 10: 1e-30
_A_UNP = (2.0 ** -11) * (10.0 / 2047.0)      # th -> x scale
_B_UNP = -2048.0 * (10.0 / 2047.0)           # th -> x offset
_Y_UNP = 10.0 / 2047.0

# ---- data pack layout (f32 cols, [96, D_W]) ----
_D_CROW = 0                          # [2,48,3] own-molecule coords, c inner
_D_CTR = 288                         # [2,3] own coords
_D_CPK = 294                         # [2,48] packed (qx,qy) exact-int f32
_D_CZ = 390                          # [2,24] -> f16 view [2,48] z coords
_D_SPM = 438                         # [2,4,48] species one-hot
_D_OH8 = 822                         # [2,4] -> f16 view [2,8] radial scatter oh
_D_W = 830

_BUILT = {}


def _f16pack(a):
    """Pack an even-length f16 row vector into f32 storage."""
    h = np.asarray(a, np.float16)
    return h.view(np.float32)


def _constpack():
    cp = np.zeros((128, _C_W), np.float32)
    cp[:, _C_JROW:_C_JROW + 48] = np.arange(48, dtype=np.float32)[None, :]
    shrow = np.repeat(np.linspace(0.9, 5.2, 17)[:-1].astype(np.float32), 48)
    cp[:, _C_SHROW:_C_SHROW + 768] = shrow[None, :]
    c1_, c3_, c5_, c7_ = [float(c) for c in SINC]
    scvals = [c7_, c5_, c3_, c1_, 1.0 / RCR, -0.5, 0.125, -0.125,
              1.0 / RCA, 0.5, 1e-30] + \
        [float(math.cos(SHFZ[z])) for z in range(8)]
    for i, v in enumerate(scvals):
        cp[:, _C_SC + i] = v
    for t in range(2):
        for m in range(4):
            o = 80 * t + 20 * m
            # per-slot LW term is -2*(d - 2*shfa)^2 + lnfc + (4 shfa^2 + ln2/2)
            cp[:, _C_SHFA + o:_C_SHFA + o + 20] = float(2.0 * SHFA[m])
            cp[:, _C_BW + o:_C_BW + o + 20] = \
                float(4.0 * SHFA[m] ** 2 + LN2H)
    for z in range(8):
        cp[:, _C_BZ + z] = 64.0 * math.log(math.cos(SHFZ[z] / 2))
    cp[:, _C_B975] = 0.975
    cp[:, _C_B025] = 0.025
    # expd (f16, +1/-1 for slot a/b of each pair) at bases 0,32,64
    expd = np.zeros((K, NPAIR), np.float32)
    exps = np.zeros((K, NPAIR), np.float32)
    expd[IA, np.arange(NPAIR)] = 1.0
    expd[IB, np.arange(NPAIR)] -= 1.0
    exps[IA, np.arange(NPAIR)] = 1.0
    exps[IB, np.arange(NPAIR)] += 1.0
    expd16 = _f16pack(expd)                  # (K, 95) f32 view of f16 data
    exps16 = _f16pack(exps)
    for b in (0, 32, 64):
        cp[b:b + K, _C_EXPD16:_C_EXPD16 + 95] = expd16
        cp[b:b + K, _C_EXPS16:_C_EXPS16 + 95] = exps16
    for b in (0, 32, 64, 96):
        cp[b:b + K, _C_EXPS:_C_EXPS + 190] = exps
    # bucket one-hot (f16): rows = pair-in-chunk, cols = ci*32 + p (pad to 32)
    triu = np.zeros((4, 4), np.int64)
    s1, s2 = np.triu_indices(4)
    triu[s1, s2] = np.arange(10); triu[s2, s1] = np.arange(10)
    slot_sp = np.repeat(np.arange(4), 5)
    pair_p = triu[slot_sp[IA], slot_sp[IB]]
    buck = np.zeros((CW, 64), np.float16)
    for ci in range(2):
        oh = (pair_p[CW * ci:CW * ci + CW, None] == np.arange(10))
        buck[:, 32 * ci:32 * ci + 10] = oh.astype(np.float16)
    cp[:CW, _C_BUCK:_C_BUCK + 32] = buck.view(np.float32)
    cp[:96, _C_IDT16:_C_IDT16 + 48] = np.eye(96, dtype=np.float16).view(np.float32)
    cp[:96, _C_IDT32:_C_IDT32 + 96] = np.eye(96, dtype=np.float32)
    return cp


def _build():
    import sys
    sys.path.insert(0, "/opt/trn_rl_repo")
    from contextlib import ExitStack
    import concourse.tile as tile
    from concourse import bacc, mybir

    f32 = mybir.dt.float32
    f16 = mybir.dt.float16
    u32 = mybir.dt.uint32
    AF = mybir.ActivationFunctionType
    ALU = mybir.AluOpType
    AX = mybir.AxisListType

    nc = bacc.Bacc("TRN2", target_bir_lowering=False, debug=False, num_devices=8)
    P_data = nc.declare_dram_parameter("data", [96, _D_W], f32, isOutput=False)
    P_const = nc.declare_dram_parameter("consts", [128, _C_W], f32, isOutput=False)
    P_outr = nc.declare_dram_parameter("outr", [8, 1536], f16, isOutput=True)
    P_outa = nc.declare_dram_parameter("outa", [96, 2304], f16, isOutput=True)

    with tile.TileContext(nc) as tc, ExitStack() as ctx:
        pool = ctx.enter_context(tc.tile_pool(name="sb", bufs=1))
        psum = ctx.enter_context(tc.tile_pool(name="ps", bufs=1, space="PSUM"))

        def sbuf(shape, tag, dt=f32):
            return pool.tile(shape, dt, name=tag, tag=tag)

        _bank_n = [0]

        def pbank(p0, p1, cols, dt=f32):
            _bank_n[0] += 1
            tl = psum.tile([128, 512], dt, name=f"bank{_bank_n[0]}",
                           tag="bank", bufs=5)
            return tl[p0:p1, 0:cols]

        CF = sbuf([128, _C_W], "constf")
        data = sbuf([96, _D_W], "data")
        nc.sync.dma_start(data[:, 0:_D_CPK], P_data[:, 0:_D_CPK])
        nc.sync.dma_start(data[:, _D_CPK:_D_W], P_data[:, _D_CPK:_D_W])
        nc.sync.dma_start(CF[:, 0:_C_EXPD16], P_const[:, 0:_C_EXPD16])
        nc.sync.dma_start(CF[:, _C_EXPD16:_C_W], P_const[:, _C_EXPD16:_C_W])
        # pin the combined Ln+Exp act table (Square/Copy are in every set)
        from concourse.hw_specs import get_activation_tables
        _tables = list(get_activation_tables(nc.m.arch).keys())
        _set_id = _tables.index("natural_log_exp_and_others")
        nc.scalar.add_instruction(mybir.InstLoadActFuncSet(
            name=nc.get_next_instruction_name(), ins=[], outs=[],
            act_func_set_id=_set_id))

        jrow = CF[0:96, _C_JROW:_C_JROW + 48]
        shrow = CF[0:96, _C_SHROW:_C_SHROW + 768]

        def ccol(i, n, w):
            return CF[0:n, _C_SC + i:_C_SC + i + 1].rearrange(
                "p (o c) -> p o c", o=1).broadcast_to([n, 1, w])

        def pool_affine(dst3, src3, imul, iadd, n, w):
            nc.gpsimd.tensor_tensor(dst3, src3, ccol(imul, n, w), ALU.mult)
            nc.gpsimd.tensor_tensor(dst3, dst3, ccol(iadd, n, w), ALU.add)

        def poly_sin(dst, z2buf, zbuf, tmp, n, w):
            # Pool-engine sin(pi*z) poly: (((c7*z2+c5)*z2+c3)*z2+c1)*z
            t3 = tmp.rearrange("p (o c) -> p o c", o=1)
            z23 = z2buf.rearrange("p (o c) -> p o c", o=1)
            pool_affine(t3, z23, 0, 1, n, w)
            nc.gpsimd.tensor_tensor(t3, t3, z23, ALU.mult)
            nc.gpsimd.tensor_tensor(t3, t3, ccol(2, n, w), ALU.add)
            nc.gpsimd.tensor_tensor(t3, t3, z23, ALU.mult)
            nc.gpsimd.tensor_tensor(t3, t3, ccol(3, n, w), ALU.add)
            nc.gpsimd.tensor_tensor(dst.rearrange("p (o c) -> p o c", o=1),
                                    t3, zbuf.rearrange("p (o c) -> p o c", o=1),
                                    ALU.mult)

        crow = data[:, _D_CROW:_D_CROW + 288].rearrange(
            "p (t j c) -> p t j c", t=2, c=3)
        ctr = data[:, _D_CTR:_D_CTR + 6].rearrange("p (t c) -> p t c", t=2)
        cpk = data[:, _D_CPK:_D_CPK + 96].rearrange("p (t j) -> p t j", t=2)
        cz16 = data[:, _D_CZ:_D_CZ + 48].bitcast(f16).rearrange(
            "p (t j) -> p t j", t=2)
        spm = data[:, _D_SPM:_D_SPM + 384].rearrange(
            "p (t s j) -> p t s j", t=2, s=4)
        oh16 = data[:, _D_OH8:_D_OH8 + 8].bitcast(f16).rearrange(
            "p (t q) -> p t q", t=2)

        # ---------------- distances ----------------
        diff = sbuf([96, 288], "diff")
        d3 = diff[:].rearrange("p (t j c) -> p t j c", t=2, c=3)
        ctr_b = data[:, _D_CTR:_D_CTR + 6].rearrange(
            "p (t o c) -> p t o c", t=2, o=1).broadcast_to([96, 2, 48, 3])
        nc.vector.tensor_tensor(d3, crow, ctr_b, ALU.subtract)
        sqd = sbuf([96, 288], "sqd")
        nc.scalar.activation(sqd[:], diff[:], AF.Square, bias=0.0, scale=1.0)
        d2 = sbuf([96, 96], "d2")
        nc.vector.tensor_reduce(
            d2[:].rearrange("p (t j o) -> p t j o", t=2, o=1),
            sqd[:].rearrange("p (t j c) -> p t j c", t=2, c=3), AX.X, ALU.add)
        # no max-guard needed: Ln(0) -> -inf -> exp -> dr=0 for self pairs,
        # which every consumer already masks or tolerates
        ln2 = sbuf([96, 96], "ln2")
        dr = sbuf([96, 96], "dr")
        nc.scalar.activation(ln2[:], d2[:], AF.Ln, bias=0.0, scale=1.0)
        nc.scalar.activation(dr[:], ln2[:], AF.Exp, bias=0.0, scale=0.5)
        selfm = sbuf([96, 96], "selfm")
        nc.vector.tensor_scalar(selfm[:], d2[:], EPS2, None, ALU.is_ge, ALU.bypass)

        # ---------------- radial ----------------
        zz = sbuf([96, 96], "zz")
        z2 = sbuf([96, 96], "z2")
        h = sbuf([96, 96], "h")
        ptmp = sbuf([96, 96], "ptmp")
        nc.vector.tensor_scalar(zz[:], dr[:], RCR, None, ALU.min, ALU.bypass)
        zz3 = zz[:].rearrange("p (o c) -> p o c", o=1)
        pool_affine(zz3, zz3, 4, 5, 96, 96)
        nc.gpsimd.tensor_tensor(z2[:], zz[:], zz[:], ALU.mult)
        poly_sin(h[:], z2[:], zz[:], ptmp[:], 96, 96)
        wr = sbuf([96, 96], "wr")
        wr3 = wr[:].rearrange("p (o c) -> p o c", o=1)
        pool_affine(wr3, h[:].rearrange("p (o c) -> p o c", o=1), 7, 6, 96, 96)
        nc.gpsimd.tensor_tensor(wr[:], wr[:], selfm[:], ALU.mult)
        wrb = sbuf([96, 96], "wrb", f16)
        nc.gpsimd.tensor_copy(wrb[:], wr[:])

        rp = sbuf([96, 1536], "rp")
        rp4 = rp[:].rearrange("p (t f j) -> p t f j", t=2, f=16)
        dr_b = dr[:].rearrange("p (t o j) -> p t o j", t=2, o=1).broadcast_to(
            [96, 2, 16, 48])
        sh_b = shrow.rearrange("p (o f j) -> p o f j", o=1, f=16).broadcast_to(
            [96, 2, 16, 48])
        # big radial subtract fully on Pool (3D views per t)
        sh_b1 = shrow.rearrange("p (f j) -> p f j", f=16)
        for t in range(2):
            dr_bt = dr[:, 48 * t:48 * t + 48].rearrange(
                "p (o j) -> p o j", o=1).broadcast_to([96, 16, 48])
            nc.gpsimd.tensor_tensor(rp4[:, t, :, :], dr_bt, sh_b1,
                                    ALU.subtract)
        nc.scalar.activation(rp[:], rp[:], AF.Square, bias=0.0, scale=1.0)
        rpb = sbuf([96, 1536], "rpb", f16)
        nc.scalar.activation(rpb[:], rp[:], AF.Exp, bias=0.0, scale=-ETA_R)
        rpb4 = rpb[:].rearrange("p (t f j) -> p t f j", t=2, f=16)
        wr_b = wrb[:].rearrange("p (t o j) -> p t o j", t=2, o=1).broadcast_to(
            [96, 2, 16, 48])
        nc.vector.tensor_tensor(rpb4, rpb4, wr_b, ALU.mult)
        outr_sb = sbuf([8, 1536], "outrsb", f16)
        for t in range(2):
            for half in range(2):
                rps = pbank(0, 8, 384)
                rhs = rpb[:].rearrange("p (t f j) -> p t f j", t=2, f=16)[
                    :, t, 8 * half:8 * half + 8, :]
                nc.tensor.matmul(rps, oh16[:, t, :], rhs, start=True, stop=True)
                dst = outr_sb[:, 768 * t + 384 * half:768 * t + 384 * half + 384]
                if half == 0:
                    nc.scalar.copy(dst, rps)
                else:
                    nc.vector.tensor_copy(dst, rps)
        nc.sync.dma_start(P_outr[:, :], outr_sb[:])

        # ---------------- neighbor selection ----------------
        # key w = relu(RCA^2 - d2) * selfm: <=0 outside cutoff, 0 for self.
        # Invalid (empty) slots then decode to d=RCA where fc=0 kills their
        # weight (wm <= ~e^-12) -- no explicit valid mask needed.
        w0 = sbuf([96, 96], "w0")
        w = sbuf([96, 96], "w")
        nc.vector.tensor_scalar(w0[:], d2[:], -1.0, CSEL, ALU.mult, ALU.add)
        nc.vector.scalar_tensor_tensor(w[:], w0[:], 0.0, selfm[:],
                                       ALU.max, ALU.mult)
        keys = sbuf([96, 384], "keys")
        k4 = keys[:].rearrange("p (t s j) -> p t s j", t=2, s=4)
        for t in range(2):
            w_bt = w[:, 48 * t:48 * t + 48].rearrange(
                "p (o j) -> p o j", o=1).broadcast_to([96, 4, 48])
            nc.gpsimd.tensor_tensor(k4[:, t, :, :], spm[:, t, :, :], w_bt,
                                    ALU.mult)
        mv8 = sbuf([96, 64], "mv8")
        mv4 = mv8[:].rearrange("p (t s q) -> p t s q", t=2, s=4)
        for t in range(2):
            for s in range(4):
                nc.vector.max(mv4[:, t, s, :], k4[:, t, s, :])
        mvc = sbuf([96, 40], "mvc")
        nc.vector.tensor_copy(mvc[:].rearrange("p (t s q) -> p t s q", t=2, s=4),
                              mv4[:, :, :, 0:5])
        d2s = sbuf([96, 40], "d2s")
        nc.vector.tensor_scalar(d2s[:], mvc[:], -1.0, CSEL, ALU.mult, ALU.add)
        lnd = sbuf([96, 40], "lnd")
        ivd = sbuf([96, 40], "ivd")
        nc.scalar.activation(lnd[:], d2s[:], AF.Ln, bias=0.0, scale=1.0)
        nc.scalar.activation(ivd[:], lnd[:], AF.Exp, bias=0.0, scale=-0.5)

        # slot-space channel tiles
        stileB = sbuf([96, 192], "stileB")       # [2t, 96]: S@0, SQ2@32, LW3@64
        lwall = sbuf([96, 192], "lwall")         # [2t, 3m, 32]: LW0-2@0:20
        utile = sbuf([96, 192], "utile", f16)    # [2t, 96]: ux@0, uy@32, uz@64
        nc.gpsimd.memset(stileB[:], 0.0)
        nc.gpsimd.memset(lwall[:], 0.0)
        nc.gpsimd.memset(utile[:], 0.0)
        sB = stileB[:].rearrange("p (t c) -> p t c", t=2)
        lw4 = lwall[:].rearrange("p (t m c) -> p t m c", t=2, m=3)
        ut = utile[:].rearrange("p (t c) -> p t c", t=2)
        dsc = sbuf([96, 40], "dsc")              # compact d per slot
        nc.scalar.activation(dsc[:], lnd[:], AF.Exp, bias=0.0, scale=0.5)
        nc.vector.tensor_copy(sB[:, :, 0:20],
                              dsc[:].rearrange("p (t k) -> p t k", t=2))
        nc.vector.tensor_scalar(sB[:, :, 32:52],
                                d2s[:].rearrange("p (t k) -> p t k", t=2),
                                2.0, None, ALU.mult, ALU.bypass)

        # ---------------- gather ----------------
        # select-mask by key-value equality: mask[p,(t,k),j] = (w[p,t,j]==mvc)
        # (f32 ties are ~never; empty slots match many j -> garbage u, killed
        #  by vkill downstream, same as the index path)
        i33 = sbuf([96, 1920], "i33", f16)
        i3m = i33[:].rearrange("p (t k j) -> p t k j", t=2, j=48)
        mv_b = mvc[:].rearrange("p (t k o) -> p t k o", t=2, o=1).broadcast_to(
            [96, 2, 20, 48])
        w_b2 = w[:].rearrange("p (t o j) -> p t o j", t=2, o=1).broadcast_to(
            [96, 2, 20, 48])
        nc.vector.tensor_tensor(i3m, w_b2, mv_b, ALU.is_equal)
        # z channel first: its u-ops run on Pool, overlapping the xy path
        i3t = i33[:].rearrange("p (t k j) -> p t k j", t=2, j=48)
        gz = sbuf([96, 1920], "gz", f16)
        cz_b = data[:, _D_CZ:_D_CZ + 48].bitcast(f16).rearrange(
            "p (t o j) -> p t o j", t=2, o=1).broadcast_to([96, 2, 20, 48])
        nc.vector.tensor_tensor(
            gz[:].rearrange("p (t k j) -> p t k j", t=2, j=48), i3t, cz_b,
            ALU.mult)
        zg = sbuf([96, 40], "zg")
        nc.vector.tensor_reduce(zg[:].rearrange("p (tk o) -> p tk o", o=1),
                                gz[:].rearrange("p (tk j) -> p tk j", j=48),
                                AX.X, ALU.add)
        dxz = sbuf([96, 40], "dxz")
        dxz2 = dxz[:].rearrange("p (t k) -> p t k", t=2)
        ctrz = ctr[:, :, 2:3].broadcast_to([96, 2, 20])
        nc.gpsimd.tensor_tensor(dxz2, zg[:].rearrange("p (t k) -> p t k", t=2),
                                ctrz, ALU.subtract)
        nc.gpsimd.tensor_tensor(ut[:, :, 64:84], dxz2,
                                ivd[:].rearrange("p (t k) -> p t k", t=2),
                                ALU.mult)
        gp = sbuf([96, 1920], "gp")
        cpk_b = data[:, _D_CPK:_D_CPK + 96].rearrange(
            "p (t o j) -> p t o j", t=2, o=1).broadcast_to([96, 2, 20, 48])
        nc.vector.tensor_tensor(
            gp[:].rearrange("p (t k j) -> p t k j", t=2, j=48), i3t, cpk_b,
            ALU.mult)
        g = sbuf([96, 40], "g")
        nc.vector.tensor_reduce(g[:].rearrange("p (tk o) -> p tk o", o=1),
                                gp[:].rearrange("p (tk j) -> p tk j", j=48),
                                AX.X, ALU.add)
        # unpack packed xy via f32 round-to-int trick:
        #   A = g*2^-11 - (0.5 - 2^-12); qxp = (A + 2^23) - 2^23 = qx + 2^11
        #   qy = g - qxp*2^11
        qy = sbuf([96, 40], "qy")
        th = sbuf([96, 40], "th")
        xr = sbuf([96, 40], "xr")
        yr = sbuf([96, 40], "yr")
        _c = 0.5 - 2.0 ** -12
        nc.vector.tensor_scalar(th[:], g[:], 2.0 ** -11, -_c, ALU.mult, ALU.add)
        nc.vector.tensor_scalar(th[:], th[:], 2.0 ** 23, -(2.0 ** 23),
                                ALU.add, ALU.add)
        nc.vector.scalar_tensor_tensor(qy[:], th[:], -(2.0 ** 11), g[:],
                                       ALU.mult, ALU.add)
        # x = (qxp - 2^11) * 10/2047 = qxp*Y - 2^11*Y
        nc.vector.tensor_scalar(xr[:], th[:], _Y_UNP, -(2.0 ** 11) * _Y_UNP,
                                ALU.mult, ALU.add)
        nc.vector.tensor_scalar(yr[:], qy[:], _Y_UNP, None, ALU.mult, ALU.bypass)
        # u channels: (coord - ctr) * ivd -> utile f16 (written in place)
        for c, src in ((0, xr), (1, yr)):
            dx = sbuf([96, 40], f"dx{c}")
            dx2 = dx[:].rearrange("p (t k) -> p t k", t=2)
            ctrc = ctr[:, :, c:c + 1].broadcast_to([96, 2, 20])
            nc.vector.tensor_tensor(dx2,
                                    src[:].rearrange("p (t k) -> p t k", t=2),
                                    ctrc, ALU.subtract)
            nc.vector.tensor_tensor(ut[:, :, 32 * c:32 * c + 20], dx2,
                                    ivd[:].rearrange("p (t k) -> p t k", t=2),
                                    ALU.mult)

        # ---------------- fc_a + LW channels ----------------
        az = sbuf([96, 40], "az")
        az2 = sbuf([96, 40], "az2")
        ah = sbuf([96, 40], "ah")
        aptmp = sbuf([96, 40], "aptmp")
        nc.vector.tensor_scalar(az[:], dsc[:], RCA, None, ALU.min, ALU.bypass)
        az3 = az[:].rearrange("p (o c) -> p o c", o=1)
        pool_affine(az3, az3, 8, 5, 96, 40)
        nc.gpsimd.tensor_tensor(az2[:], az[:], az[:], ALU.mult)
        poly_sin(ah[:], az2[:], az[:], aptmp[:], 96, 40)
        kh = sbuf([96, 40], "kh")
        kh3 = kh[:].rearrange("p (o c) -> p o c", o=1)
        nc.gpsimd.tensor_tensor(kh3, ah[:].rearrange("p (o c) -> p o c", o=1),
                                ccol(5, 96, 40), ALU.mult)
        nc.gpsimd.tensor_tensor(kh3, kh3, ccol(9, 96, 40), ALU.add)
        nc.vector.tensor_scalar(kh[:], kh[:], 1e-30, None, ALU.max, ALU.bypass)
        lnfc = sbuf([96, 40], "lnfc")
        nc.scalar.activation(lnfc[:], kh[:], AF.Ln, bias=0.0, scale=1.0)
        # LW_m = -2*(ds - shfa_m)^2 + lnfc + bw_m/2  (batched over m, t outer)
        am = sbuf([96, 160], "am")
        am4 = am[:].rearrange("p (t m k) -> p t m k", t=2, m=4)
        ds_b = dsc[:].rearrange("p (t o k) -> p t o k", t=2, o=1).broadcast_to(
            [96, 2, 4, 20])
        shfarow = CF[0:96, _C_SHFA:_C_SHFA + 160].rearrange(
            "p (t m k) -> p t m k", t=2, m=4)
        nc.vector.tensor_tensor(am4, ds_b, shfarow, ALU.subtract)
        nc.vector.scalar_tensor_tensor(am[:], am[:], -2.0, am[:],
                                       ALU.mult, ALU.mult)
        lnfc_b = lnfc[:].rearrange("p (t o k) -> p t o k", t=2, o=1).broadcast_to(
            [96, 2, 4, 20])
        bwrow = CF[0:96, _C_BW:_C_BW + 160].rearrange(
            "p (t m k) -> p t m k", t=2, m=4)
        lnb = sbuf([96, 160], "lnb")
        lnb4 = lnb[:].rearrange("p (t m k) -> p t m k", t=2, m=4)
        nc.vector.tensor_tensor(lnb4, lnfc_b, bwrow, ALU.add)
        nc.vector.tensor_tensor(lw4[:, :, :, 0:20], am4[:, :, 0:3, :],
                                lnb4[:, :, 0:3, :], ALU.add)
        nc.vector.tensor_tensor(sB[:, :, 64:84], am4[:, :, 3, :],
                                lnb4[:, :, 3, :], ALU.add)

        # ---------------- hi/lo f16 split of f32 slot channels ----------------
        # v = hi + lo reconstructs f32 precision; the pair matmuls then run as
        # two accumulating f16 matmuls (1 cyc/row) instead of one f32 (4 cyc)
        lwH = sbuf([96, 192], "lwH", f16)
        lwL = sbuf([96, 192], "lwL", f16)
        sbH = sbuf([96, 192], "sbH", f16)
        sbL = sbuf([96, 192], "sbL", f16)
        nc.scalar.copy(lwH[:], lwall[:])
        nc.vector.tensor_tensor(lwL[:], lwall[:], lwH[:], ALU.subtract)
        nc.scalar.copy(sbH[:], stileB[:])
        nc.vector.tensor_tensor(sbL[:], stileB[:], sbH[:], ALU.subtract)

        # ---------------- transposes to SD ----------------
        # hi and lo transposed into one psum bank -> one copy per (src, t)
        idt16 = CF[0:96, _C_IDT16:_C_IDT16 + 48].bitcast(f16)
        SDu = sbuf([96, 192], "sdu", f16)      # ux@0, uy@32, uz@64
        SDlw = sbuf([96, 384], "sdlw", f16)    # [hl, at]: LW0@0, LW1@32, LW2@64
        SDb = sbuf([96, 384], "sdb", f16)      # [hl, at]: S@0, SQ2@32, LW3@64
        for t in range(2):
            tp = pbank(0, 96, 96, dt=f16)
            nc.tensor.transpose(tp, ut[:, t, :], idt16)
            nc.vector.tensor_copy(SDu[:, 96 * t:96 * t + 96], tp)
            for hi_, lo_, dst_, eng in ((lwH, lwL, SDlw, nc.scalar),
                                        (sbH, sbL, SDb, nc.vector)):
                tpx = pbank(0, 96, 192, dt=f16)
                nc.tensor.transpose(tpx[:, 0:96], hi_[:, 96 * t:96 * t + 96],
                                    idt16)
                nc.tensor.transpose(tpx[:, 96:192], lo_[:, 96 * t:96 * t + 96],
                                    idt16)
                dv = dst_[:].rearrange("p (hl c) -> p hl c", hl=2)[
                    :, :, 96 * t:96 * t + 96]
                sv = tpx[:, 0:192].rearrange("p (hl c) -> p hl c", hl=2)
                if eng is nc.scalar:
                    eng.copy(dv, sv)
                else:
                    eng.tensor_copy(dv, sv)

        # ---------------- pair space ----------------
        expd16 = CF[:, _C_EXPD16:_C_EXPD16 + 95].bitcast(f16)
        exps16 = CF[:, _C_EXPS16:_C_EXPS16 + 95].bitcast(f16)
        buck16 = CF[0:CW, _C_BUCK:_C_BUCK + 32].bitcast(f16)
        shh2 = sbuf([CW, 384], "shh2")
        b4 = sbuf([CW, 384], "b4")
        lwb = sbuf([CW, 1536], "lwb")
        lwb4 = lwb[:].rearrange("p (m c) -> p m c", m=4)
        for ci in range(2):
            c0 = CW * ci
            cs = slice(192 * ci, 192 * ci + 192)
            vd = [pbank(0, CW, 192) for _ in range(3)]
            for c in range(3):
                nc.tensor.matmul(vd[c], expd16[32 * c:32 * c + K, c0:c0 + CW],
                                 SDu[32 * c:32 * c + K, :], start=True,
                                 stop=True)

            def hilo_mm(bank, base, src16):
                lhsT = exps16[base:base + K, c0:c0 + CW]
                for hl in range(2):
                    nc.tensor.matmul(bank, lhsT,
                                     src16[base:base + K,
                                           192 * hl:192 * hl + 192],
                                     start=(hl == 0), stop=(hl == 1))

            pS = pbank(0, CW, 192)
            pQ = pbank(0, CW, 192)
            hilo_mm(pS, 0, SDb)
            hilo_mm(pQ, 32, SDb)
            pL = [pbank(0, CW, 192) for _ in range(4)]
            for m in range(3):
                hilo_mm(pL[m], 32 * m, SDlw)
            hilo_mm(pL[3], 64, SDb)
            # shh2 = sum_c vd_c^2
            tq = sbuf([CW, 192], f"tq{ci}")
            tq2 = sbuf([CW, 192], f"tq2{ci}")
            nc.scalar.activation(shh2[:, cs], vd[0], AF.Square, bias=0.0,
                                 scale=1.0)
            nc.scalar.activation(tq[:], vd[1], AF.Square, bias=0.0, scale=1.0)
            nc.scalar.activation(tq2[:], vd[2], AF.Square, bias=0.0, scale=1.0)
            nc.vector.tensor_tensor(shh2[:, cs], shh2[:, cs], tq[:], ALU.add)
            nc.vector.tensor_tensor(shh2[:, cs], shh2[:, cs], tq2[:], ALU.add)
            # per-chunk clip so lnc/chp don't wait for the lwb adds below
            nc.vector.tensor_scalar(shh2[:, cs], shh2[:, cs], 0.0, 4.0,
                                    ALU.max, ALU.min)
            # b4 = -2*S^2 + SQ2'
            s2 = sbuf([CW, 192], f"s2{ci}")
            nc.scalar.activation(s2[:], pS, AF.Square, bias=0.0, scale=1.0)
            nc.vector.scalar_tensor_tensor(b4[:, cs], s2[:], -2.0, pQ,
                                           ALU.mult, ALU.add)
            for m in range(4):
                nc.vector.tensor_tensor(lwb4[:, m, cs], pL[m], b4[:, cs],
                                        ALU.add)
        lnc = sbuf([CW, 384], "lnc")
        lns = sbuf([CW, 384], "lns")
        chp = sbuf([CW, 384], "chp")
        shp = sbuf([CW, 384], "shp")
        nc.scalar.activation(lnc[:], shh2[:], AF.Ln,
                             bias=CF[0:CW, _C_B975:_C_B975 + 1], scale=-0.2375)
        nc.scalar.activation(chp[:], lnc[:], AF.Exp, bias=0.0, scale=0.5)
        nc.scalar.activation(lns[:], shh2[:], AF.Ln,
                             bias=CF[0:CW, _C_B025:_C_B025 + 1], scale=0.2375)
        nc.scalar.activation(shp[:], lns[:], AF.Exp, bias=0.0, scale=0.5)
        wm = sbuf([CW, 1536], "wm", f16)
        wmv = wm[:].rearrange("p (m c) -> p m c", m=4)
        for ci in range(2):
            cc = slice(192 * ci, 192 * ci + 192)
            nc.scalar.activation(wmv[:, :, cc], lwb4[:, :, cc], AF.Exp,
                                 bias=0.0, scale=1.0)
        # tb_z = shp*tan(shfz/2) + chp  (f32), half-batched Ln, per-z biased Exp
        tb = sbuf([CW, 3072], "tb")
        tb8 = tb[:].rearrange("p (z c) -> p z c", z=8)
        for z in range(8):
            tanz = float(np.tan(SHFZ[z] / 2))
            nc.vector.scalar_tensor_tensor(tb8[:, z, :], shp[:], tanz, chp[:],
                                           ALU.mult, ALU.add)
        lnv = sbuf([CW, 3072], "lnv")
        nc.scalar.activation(lnv[:, 0:1536], tb[:, 0:1536], AF.Ln, bias=0.0,
                             scale=1.0)
        nc.scalar.activation(lnv[:, 1536:3072], tb[:, 1536:3072], AF.Ln,
                             bias=0.0, scale=1.0)
        f1 = sbuf([CW, 3072], "f1", f16)
        f18 = f1[:].rearrange("p (z c) -> p z c", z=8)
        lnv8 = lnv[:].rearrange("p (z c) -> p z c", z=8)
        # terms laid out z-major: terms[p, z, m, c] = wm[p, m, c] * f1[p, z, c]
        # flat index f = z*4 + m; bucket group g covers f in {2g, 2g+1}
        terms = sbuf([CW, 12288], "terms", f16)
        t4 = terms[:].rearrange("p (z m c) -> p z m c", z=8, m=4)
        wm4 = wm[:].rearrange("p (m c) -> p m c", m=4)
        outa_sb = sbuf([96, 2304], "outasb", f16)
        outa_banks = []
        for z in range(8):
            nc.scalar.activation(f18[:, z, :], lnv8[:, z, :], AF.Exp,
                                 bias=CF[0:CW, _C_BZ + z:_C_BZ + z + 1],
                                 scale=64.0)
            f1_b = f18[:, z, :].rearrange("p (o c) -> p o c", o=1).broadcast_to(
                [CW, 4, 384])
            nc.vector.tensor_tensor(t4[:, z, :, :], wm4, f1_b, ALU.mult)
            # two bucket groups per z (m 0-1 and m 2-3), 3 groups per psum bank
            for half in range(2):
                gidx = 2 * z + half
                bank_i, slot = divmod(gidx, 3)
                if slot == 0:
                    outa_banks.append(psum.tile(
                        [128, 512], f32, name=f"obank{bank_i}",
                        tag="obank", bufs=2))
                pA = outa_banks[bank_i][32 * slot:32 * slot + 32, 0:384]
                for ci in range(2):
                    nc.tensor.matmul(
                        pA, buck16[:, 32 * ci:32 * ci + 32],
                        t4[:, z, 2 * half:2 * half + 2, 192 * ci:192 * ci + 192],
                        start=(ci == 0), stop=(ci == 1))
                if slot == 2 or gidx == 15:
                    np_ = 32 * (slot + 1)
                    dst = outa_sb[0:np_, 384 * bank_i:384 * bank_i + 384]
                    src = outa_banks[bank_i][0:np_, 0:384]
                    if bank_i < 4:
                        nc.scalar.copy(dst, src)
                    else:
                        nc.vector.tensor_copy(dst, src)
                if gidx == 8:
                    nc.sync.dma_start(P_outa[:, 0:1152], outa_sb[:, 0:1152])
                elif gidx == 14:
                    nc.sync.dma_start(P_outa[:, 1152:1920],
                                      outa_sb[:, 1152:1920])
        nc.sync.dma_start(P_outa[0:32, 1920:2304], outa_sb[0:32, 1920:2304])

    nc.compile()
    return nc


def _pack_inputs(species, coords):
    sp = species.astype(np.int64)
    co = coords.astype(np.float32)
    cp = _BUILT["cp"]
    qx = np.round(co[..., 0] * QS)
    qy = np.round(co[..., 1] * QS)
    packed = (2.0 ** 22 + qx * 2048.0 + qy).astype(np.float32)
    z16 = co[..., 2].astype(np.float16)
    in_maps = []
    for c in range(8):
        data = np.zeros((96, _D_W), np.float32)
        oh = np.zeros((96, 2, 8), np.float16)
        for t in range(2):
            for m in range(2):
                mol = 4 * c + 2 * t + m
                rows = slice(48 * m, 48 * m + 48)
                data[rows, _D_CROW + 288 * 0:_D_CROW + 288][
                    :, 144 * t:144 * t + 144] = co[mol].reshape(-1)[None, :]
                data[rows, _D_CTR + 3 * t:_D_CTR + 3 * t + 3] = co[mol]
                data[rows, _D_CPK + 48 * t:_D_CPK + 48 * t + 48] = \
                    packed[mol][None, :]
                zv = z16[mol].view(np.float32)
                data[rows, _D_CZ + 24 * t:_D_CZ + 24 * t + 24] = zv[None, :]
                for s in range(4):
                    data[rows, _D_SPM + 192 * t + 48 * s:
                         _D_SPM + 192 * t + 48 * s + 48] = \
                        (sp[mol] == s).astype(np.float32)[None, :]
                for s in range(4):
                    oh[rows, t, 4 * m + s] = (sp[mol] == s).astype(np.float16)
        data[:, _D_OH8:_D_OH8 + 8] = oh.reshape(96, 16).view(np.float32)
        in_maps.append(dict(data=data, consts=cp))
    return in_maps


def kernel(species, coordinates):
    import sys
    sys.path.insert(0, "/opt/trn_rl_repo")
    from concourse.bass_utils import run_bass_kernel_spmd

    species = np.asarray(species)
    coords = np.asarray(coordinates, dtype=np.float32)
    N = species.shape[0]
    if "nc" not in _BUILT:
        _BUILT["cp"] = _constpack()
        _BUILT["nc"] = _build()
    nc = _BUILT["nc"]

    in_maps = _pack_inputs(species, coords)
    res = run_bass_kernel_spmd(nc, in_maps, list(range(8)))
    full = np.zeros((N, 48, 384), np.float32)
    for c in range(8):
        outr = np.asarray(res.results[c]["outr"]).astype(np.float32)
        outa = np.asarray(res.results[c]["outa"]).astype(np.float32)
        # radial: outr[p=(m,s), t*768 + f*48 + j] -> mol 4c+2t+m, atom j, s*16+f
        r = outr.reshape(2, 4, 2, 16, 48)       # [m, s, t, f, j]
        rad = r.transpose(2, 0, 4, 1, 3).reshape(2, 2, 48, 64)  # [t, m, j, sf]
        for t in range(2):
            for m in range(2):
                full[4 * c + 2 * t + m, :, 0:64] = rad[t, m]
        # angular: outa[32*slot + b, 384*i + k*192 + t*96 + mm*48 + a]
        #   g = 3i + slot; z = g//2; half = g%2; m = 2*half + k
        #   feature = 64 + b*32 + m*8 + z
        a = outa.reshape(3, 32, 6, 2, 2, 2, 48)   # [slot, 32p, i, k, t, mm, a]
        a = a[:, 0:10]                             # valid buckets
        for g in range(16):
            i, slot = divmod(g, 3)
            z, half = divmod(g, 2)
            for k in range(2):
                m = 2 * half + k
                feat = 64 + np.arange(10) * 32 + m * 8 + z
                for t in range(2):
                    for mm in range(2):
                        mol = 4 * c + 2 * t + mm
                        full[mol, :, feat] = a[slot, :, i, k, t, mm, :]
    return full


# revision 114
# speedup vs baseline: 1.0052x; 1.0052x over previous
"""ANI AEV computer on 8 TRN2 NeuronCores (Bass/Tile), data-parallel over molecules.

v13 (~47.2us, vs 75.7us baseline). Key structure:
- merged-tile per-atom phase: both 96-row tiles (2 molecules each) processed
  in one instruction stream via a free-axis t dimension (engine cost depends
  only on free size, so this halves per-op overhead vs two passes)
- selection key w = relu(RCA^2 - d2)*selfmask: out-of-cutoff and empty
  slots decode to d = RCA where fc = 0 kills their weight naturally (no
  explicit valid/vkill masking ops needed)
- neighbor gather via select-mask == top-k key value (no max_index), with
  x,y packed as exact 11-bit integers in one f32 word (single mask mult +
  reduce gathers both; unpacked with the f32 round-to-int trick) and z in f16
- f16 2x-mode DVE ops wherever precision allows; f32 kept for tb (cos
  half-angle, condition number 64) and the S/SQ2/LW channel matmuls
- z-major terms layout: each terms op fires right after its per-z Exp;
  bucket matmuls pack 3 groups per PSUM bank (bases 0/32/64) so only 6
  PSUM->SBUF copies + 2 DMAs drain the angular output
- per-chunk clip so the lnc/chp/tb/Ln chain starts before the lwb adds
- emission ordered so the Pool fc_a polynomial chain runs concurrently with
  the DVE gather (az issued pre-gather; LW tail lands right after it), and
  the angular output drains in three DMA pieces issued as banks complete
- slot channels split hi/lo into f16 pairs so the pair matmuls run at
  1 cyc/row while accumulating exact f32 values in PSUM
Shards (32,48) species / (32,48,3) coords over 8 cores (4 molecules each),
returns (32,48,384) float32.
"""
import math
import numpy as np

RCR, RCA = 5.2, 3.5
ETA_R = 16.0
SHFA = np.linspace(0.9, 3.5, 5)[:-1].astype(np.float64)
SHFZ = (np.linspace(0.0, math.pi, 9)[:-1] + math.pi / 16.0).astype(np.float64)
LN2H = 0.5 * math.log(2.0)
K = 20                              # 4 species x 5 slots
IA, IB = np.triu_indices(K, 1)
NPAIR = len(IA)                     # 190
CW = 95                             # pairs per chunk
EPS2 = 0.01
CSEL = RCA * RCA                    # selection key offset: w<=0 outside cutoff
QS = 2047.0 / 10.0                  # 11-bit coordinate quantization

# fc = 0.5 - 0.5*sin(pi*(d/rc-0.5)); odd poly deg 7 for sin(pi*z), z in [-.5,.5]
_z = np.linspace(-0.5, 0.5, 20001)
SINC, *_ = np.linalg.lstsq(np.stack([_z, _z**3, _z**5, _z**7], 1),
                           np.sin(np.pi * _z), rcond=None)

# ---- const pack layout (f32 cols, [128, C_W]) ----
_C_JROW = 0                          # [128, 48] iota j
_C_SHROW = 48                        # [128, 768] radial shifts f-major
_C_SC = 816                          # [128, 24] scalar const columns
_C_SHFA = 840                        # [128, 160] shfa row (m*40+u -> SHFA[m])
_C_BW = 1000                         # [128, 160] bw_m/2 row (m*40+u)
_C_BZ = 1160                         # [128, 8] f1 exp bias 64*ln(cos(shfz/2))
_C_B975 = 1168                       # 0.975
_C_B025 = 1169                       # 0.025
_C_EXPD16 = 1170                     # [128, 95] f32 = [190] f16, at bases 0/32/64
_C_EXPS = 1265                       # [128, 190] f32 exps, at bases 0/32/64/96
_C_BUCK = 1455                       # [128, 32] f32 = [64] f16 bucket onehot (pad 32)
_C_IDT16 = 1487                      # [128, 48] f32 = [96] f16 identity
_C_IDT32 = 1535                      # [128, 96] f32 identity
_C_EXPS16 = 1631                     # [128, 95] f32 = [190] f16 exps @0/32/64
_C_W = 1726

# scalar const column values
#  0-3: sin poly c7,c5,c3,c1;  4: 1/RCR; 5: -0.5; 6: 0.125; 7: -0.125
#  8: 1/RCA; 9: 0.5; 10: 1e-30
_A_UNP = (2.0 ** -11) * (10.0 / 2047.0)      # th -> x scale
_B_UNP = -2048.0 * (10.0 / 2047.0)           # th -> x offset
_Y_UNP = 10.0 / 2047.0

# ---- data pack layout (f32 cols, [96, D_W]) ----
_D_CROW = 0                          # [2,48,3] own-molecule coords, c inner
_D_CTR = 288                         # [2,3] own coords
_D_CPK = 294                         # [2,48] packed (qx,qy) exact-int f32
_D_CZ = 390                          # [2,24] -> f16 view [2,48] z coords
_D_SPM = 438                         # [2,4,48] species one-hot
_D_OH8 = 822                         # [2,4] -> f16 view [2,8] radial scatter oh
_D_W = 830

_BUILT = {}


def _f16pack(a):
    """Pack an even-length f16 row vector into f32 storage."""
    h = np.asarray(a, np.float16)
    return h.view(np.float32)


def _constpack():
    cp = np.zeros((128, _C_W), np.float32)
    cp[:, _C_JROW:_C_JROW + 48] = np.arange(48, dtype=np.float32)[None, :]
    shrow = np.repeat(np.linspace(0.9, 5.2, 17)[:-1].astype(np.float32), 48)
    cp[:, _C_SHROW:_C_SHROW + 768] = shrow[None, :]
    c1_, c3_, c5_, c7_ = [float(c) for c in SINC]
    scvals = [c7_, c5_, c3_, c1_, 1.0 / RCR, -0.5, 0.125, -0.125,
              1.0 / RCA, 0.5, 1e-30] + \
        [float(math.cos(SHFZ[z])) for z in range(8)]
    for i, v in enumerate(scvals):
        cp[:, _C_SC + i] = v
    for t in range(2):
        for m in range(4):
            o = 80 * t + 20 * m
            # per-slot LW term is -2*(d - 2*shfa)^2 + lnfc + (4 shfa^2 + ln2/2)
            cp[:, _C_SHFA + o:_C_SHFA + o + 20] = float(2.0 * SHFA[m])
            cp[:, _C_BW + o:_C_BW + o + 20] = \
                float(4.0 * SHFA[m] ** 2 + LN2H)
    for z in range(8):
        cp[:, _C_BZ + z] = 64.0 * math.log(math.cos(SHFZ[z] / 2))
    cp[:, _C_B975] = 0.975
    cp[:, _C_B025] = 0.025
    # expd (f16, +1/-1 for slot a/b of each pair) at bases 0,32,64
    expd = np.zeros((K, NPAIR), np.float32)
    exps = np.zeros((K, NPAIR), np.float32)
    expd[IA, np.arange(NPAIR)] = 1.0
    expd[IB, np.arange(NPAIR)] -= 1.0
    exps[IA, np.arange(NPAIR)] = 1.0
    exps[IB, np.arange(NPAIR)] += 1.0
    expd16 = _f16pack(expd)                  # (K, 95) f32 view of f16 data
    exps16 = _f16pack(exps)
    for b in (0, 32, 64):
        cp[b:b + K, _C_EXPD16:_C_EXPD16 + 95] = expd16
        cp[b:b + K, _C_EXPS16:_C_EXPS16 + 95] = exps16
    for b in (0, 32, 64, 96):
        cp[b:b + K, _C_EXPS:_C_EXPS + 190] = exps
    # bucket one-hot (f16): rows = pair-in-chunk, cols = ci*32 + p (pad to 32)
    triu = np.zeros((4, 4), np.int64)
    s1, s2 = np.triu_indices(4)
    triu[s1, s2] = np.arange(10); triu[s2, s1] = np.arange(10)
    slot_sp = np.repeat(np.arange(4), 5)
    pair_p = triu[slot_sp[IA], slot_sp[IB]]
    buck = np.zeros((CW, 64), np.float16)
    for ci in range(2):
        oh = (pair_p[CW * ci:CW * ci + CW, None] == np.arange(10))
        buck[:, 32 * ci:32 * ci + 10] = oh.astype(np.float16)
    cp[:CW, _C_BUCK:_C_BUCK + 32] = buck.view(np.float32)
    cp[:96, _C_IDT16:_C_IDT16 + 48] = np.eye(96, dtype=np.float16).view(np.float32)
    cp[:96, _C_IDT32:_C_IDT32 + 96] = np.eye(96, dtype=np.float32)
    return cp


def _build():
    import sys
    sys.path.insert(0, "/opt/trn_rl_repo")
    from contextlib import ExitStack
    import concourse.tile as tile
    from concourse import bacc, mybir

    f32 = mybir.dt.float32
    f16 = mybir.dt.float16
    u32 = mybir.dt.uint32
    AF = mybir.ActivationFunctionType
    ALU = mybir.AluOpType
    AX = mybir.AxisListType

    nc = bacc.Bacc("TRN2", target_bir_lowering=False, debug=False, num_devices=8)
    P_data = nc.declare_dram_parameter("data", [96, _D_W], f32, isOutput=False)
    P_const = nc.declare_dram_parameter("consts", [128, _C_W], f32, isOutput=False)
    P_outr = nc.declare_dram_parameter("outr", [8, 1536], f16, isOutput=True)
    P_outa = nc.declare_dram_parameter("outa", [96, 2304], f16, isOutput=True)

    with tile.TileContext(nc) as tc, ExitStack() as ctx:
        pool = ctx.enter_context(tc.tile_pool(name="sb", bufs=1))
        psum = ctx.enter_context(tc.tile_pool(name="ps", bufs=1, space="PSUM"))

        def sbuf(shape, tag, dt=f32):
            return pool.tile(shape, dt, name=tag, tag=tag)

        _bank_n = [0]

        def pbank(p0, p1, cols, dt=f32):
            _bank_n[0] += 1
            tl = psum.tile([128, 512], dt, name=f"bank{_bank_n[0]}",
                           tag="bank", bufs=5)
            return tl[p0:p1, 0:cols]

        CF = sbuf([128, _C_W], "constf")
        data = sbuf([96, _D_W], "data")
        nc.sync.dma_start(data[:, 0:_D_CPK], P_data[:, 0:_D_CPK])
        nc.sync.dma_start(data[:, _D_CPK:_D_W], P_data[:, _D_CPK:_D_W])
        nc.sync.dma_start(CF[:, 0:_C_EXPD16], P_const[:, 0:_C_EXPD16])
        nc.sync.dma_start(CF[:, _C_EXPD16:_C_W], P_const[:, _C_EXPD16:_C_W])
        # pin the combined Ln+Exp act table (Square/Copy are in every set)
        from concourse.hw_specs import get_activation_tables
        _tables = list(get_activation_tables(nc.m.arch).keys())
        _set_id = _tables.index("natural_log_exp_and_others")
        nc.scalar.add_instruction(mybir.InstLoadActFuncSet(
            name=nc.get_next_instruction_name(), ins=[], outs=[],
            act_func_set_id=_set_id))

        jrow = CF[0:96, _C_JROW:_C_JROW + 48]
        shrow = CF[0:96, _C_SHROW:_C_SHROW + 768]

        def ccol(i, n, w):
            return CF[0:n, _C_SC + i:_C_SC + i + 1].rearrange(
                "p (o c) -> p o c", o=1).broadcast_to([n, 1, w])

        def pool_affine(dst3, src3, imul, iadd, n, w):
            nc.gpsimd.tensor_tensor(dst3, src3, ccol(imul, n, w), ALU.mult)
            nc.gpsimd.tensor_tensor(dst3, dst3, ccol(iadd, n, w), ALU.add)

        def poly_sin(dst, z2buf, zbuf, tmp, n, w):
            # Pool-engine sin(pi*z) poly: (((c7*z2+c5)*z2+c3)*z2+c1)*z
            t3 = tmp.rearrange("p (o c) -> p o c", o=1)
            z23 = z2buf.rearrange("p (o c) -> p o c", o=1)
            pool_affine(t3, z23, 0, 1, n, w)
            nc.gpsimd.tensor_tensor(t3, t3, z23, ALU.mult)
            nc.gpsimd.tensor_tensor(t3, t3, ccol(2, n, w), ALU.add)
            nc.gpsimd.tensor_tensor(t3, t3, z23, ALU.mult)
            nc.gpsimd.tensor_tensor(t3, t3, ccol(3, n, w), ALU.add)
            nc.gpsimd.tensor_tensor(dst.rearrange("p (o c) -> p o c", o=1),
                                    t3, zbuf.rearrange("p (o c) -> p o c", o=1),
                                    ALU.mult)

        crow = data[:, _D_CROW:_D_CROW + 288].rearrange(
            "p (t j c) -> p t j c", t=2, c=3)
        ctr = data[:, _D_CTR:_D_CTR + 6].rearrange("p (t c) -> p t c", t=2)
        cpk = data[:, _D_CPK:_D_CPK + 96].rearrange("p (t j) -> p t j", t=2)
        cz16 = data[:, _D_CZ:_D_CZ + 48].bitcast(f16).rearrange(
            "p (t j) -> p t j", t=2)
        spm = data[:, _D_SPM:_D_SPM + 384].rearrange(
            "p (t s j) -> p t s j", t=2, s=4)
        oh16 = data[:, _D_OH8:_D_OH8 + 8].bitcast(f16).rearrange(
            "p (t q) -> p t q", t=2)

        # ---------------- distances ----------------
        diff = sbuf([96, 288], "diff")
        d3 = diff[:].rearrange("p (t j c) -> p t j c", t=2, c=3)
        ctr_b = data[:, _D_CTR:_D_CTR + 6].rearrange(
            "p (t o c) -> p t o c", t=2, o=1).broadcast_to([96, 2, 48, 3])
        nc.vector.tensor_tensor(d3, crow, ctr_b, ALU.subtract)
        sqd = sbuf([96, 288], "sqd")
        nc.scalar.activation(sqd[:], diff[:], AF.Square, bias=0.0, scale=1.0)
        d2 = sbuf([96, 96], "d2")
        nc.vector.tensor_reduce(
            d2[:].rearrange("p (t j o) -> p t j o", t=2, o=1),
            sqd[:].rearrange("p (t j c) -> p t j c", t=2, c=3), AX.X, ALU.add)
        # no max-guard needed: Ln(0) -> -inf -> exp -> dr=0 for self pairs,
        # which every consumer already masks or tolerates
        ln2 = sbuf([96, 96], "ln2")
        dr = sbuf([96, 96], "dr")
        nc.scalar.activation(ln2[:], d2[:], AF.Ln, bias=0.0, scale=1.0)
        nc.scalar.activation(dr[:], ln2[:], AF.Exp, bias=0.0, scale=0.5)
        selfm = sbuf([96, 96], "selfm")
        nc.vector.tensor_scalar(selfm[:], d2[:], EPS2, None, ALU.is_ge, ALU.bypass)

        # ---------------- radial ----------------
        zz = sbuf([96, 96], "zz")
        z2 = sbuf([96, 96], "z2")
        h = sbuf([96, 96], "h")
        ptmp = sbuf([96, 96], "ptmp")
        nc.vector.tensor_scalar(zz[:], dr[:], RCR, None, ALU.min, ALU.bypass)
        zz3 = zz[:].rearrange("p (o c) -> p o c", o=1)
        pool_affine(zz3, zz3, 4, 5, 96, 96)
        nc.gpsimd.tensor_tensor(z2[:], zz[:], zz[:], ALU.mult)
        poly_sin(h[:], z2[:], zz[:], ptmp[:], 96, 96)
        wr = sbuf([96, 96], "wr")
        wr3 = wr[:].rearrange("p (o c) -> p o c", o=1)
        pool_affine(wr3, h[:].rearrange("p (o c) -> p o c", o=1), 7, 6, 96, 96)
        nc.gpsimd.tensor_tensor(wr[:], wr[:], selfm[:], ALU.mult)
        wrb = sbuf([96, 96], "wrb", f16)
        nc.gpsimd.tensor_copy(wrb[:], wr[:])

        rp = sbuf([96, 1536], "rp")
        rp4 = rp[:].rearrange("p (t f j) -> p t f j", t=2, f=16)
        dr_b = dr[:].rearrange("p (t o j) -> p t o j", t=2, o=1).broadcast_to(
            [96, 2, 16, 48])
        sh_b = shrow.rearrange("p (o f j) -> p o f j", o=1, f=16).broadcast_to(
            [96, 2, 16, 48])
        # big radial subtract fully on Pool (3D views per t)
        sh_b1 = shrow.rearrange("p (f j) -> p f j", f=16)
        for t in range(2):
            dr_bt = dr[:, 48 * t:48 * t + 48].rearrange(
                "p (o j) -> p o j", o=1).broadcast_to([96, 16, 48])
            nc.gpsimd.tensor_tensor(rp4[:, t, :, :], dr_bt, sh_b1,
                                    ALU.subtract)
        nc.scalar.activation(rp[:], rp[:], AF.Square, bias=0.0, scale=1.0)
        rpb = sbuf([96, 1536], "rpb", f16)
        nc.scalar.activation(rpb[:], rp[:], AF.Exp, bias=0.0, scale=-ETA_R)
        rpb4 = rpb[:].rearrange("p (t f j) -> p t f j", t=2, f=16)
        wr_b = wrb[:].rearrange("p (t o j) -> p t o j", t=2, o=1).broadcast_to(
            [96, 2, 16, 48])
        nc.vector.tensor_tensor(rpb4, rpb4, wr_b, ALU.mult)
        outr_sb = sbuf([8, 1536], "outrsb", f16)
        for t in range(2):
            for half in range(2):
                rps = pbank(0, 8, 384)
                rhs = rpb[:].rearrange("p (t f j) -> p t f j", t=2, f=16)[
                    :, t, 8 * half:8 * half + 8, :]
                nc.tensor.matmul(rps, oh16[:, t, :], rhs, start=True, stop=True)
                dst = outr_sb[:, 768 * t + 384 * half:768 * t + 384 * half + 384]
                if half == 0:
                    nc.scalar.copy(dst, rps)
                else:
                    nc.vector.tensor_copy(dst, rps)
        nc.sync.dma_start(P_outr[:, :], outr_sb[:])

        # ---------------- neighbor selection ----------------
        # key w = relu(RCA^2 - d2) * selfm: <=0 outside cutoff, 0 for self.
        # Invalid (empty) slots then decode to d=RCA where fc=0 kills their
        # weight (wm <= ~e^-12) -- no explicit valid mask needed.
        w0 = sbuf([96, 96], "w0")
        w = sbuf([96, 96], "w")
        nc.vector.tensor_scalar(w0[:], d2[:], -1.0, CSEL, ALU.mult, ALU.add)
        nc.vector.scalar_tensor_tensor(w[:], w0[:], 0.0, selfm[:],
                                       ALU.max, ALU.mult)
        keys = sbuf([96, 384], "keys")
        k4 = keys[:].rearrange("p (t s j) -> p t s j", t=2, s=4)
        for t in range(2):
            w_bt = w[:, 48 * t:48 * t + 48].rearrange(
                "p (o j) -> p o j", o=1).broadcast_to([96, 4, 48])
            nc.gpsimd.tensor_tensor(k4[:, t, :, :], spm[:, t, :, :], w_bt,
                                    ALU.mult)
        mv8 = sbuf([96, 64], "mv8")
        mv4 = mv8[:].rearrange("p (t s q) -> p t s q", t=2, s=4)
        for t in range(2):
            for s in range(4):
                nc.vector.max(mv4[:, t, s, :], k4[:, t, s, :])
        mvc = sbuf([96, 40], "mvc")
        nc.vector.tensor_copy(mvc[:].rearrange("p (t s q) -> p t s q", t=2, s=4),
                              mv4[:, :, :, 0:5])
        d2s = sbuf([96, 40], "d2s")
        nc.vector.tensor_scalar(d2s[:], mvc[:], -1.0, CSEL, ALU.mult, ALU.add)
        lnd = sbuf([96, 40], "lnd")
        ivd = sbuf([96, 40], "ivd")
        nc.scalar.activation(lnd[:], d2s[:], AF.Ln, bias=0.0, scale=1.0)
        nc.scalar.activation(ivd[:], lnd[:], AF.Exp, bias=0.0, scale=-0.5)

        # slot-space channel tiles
        stileB = sbuf([96, 192], "stileB")       # [2t, 96]: S@0, SQ2@32, LW3@64
        lwall = sbuf([96, 192], "lwall")         # [2t, 3m, 32]: LW0-2@0:20
        utile = sbuf([96, 192], "utile", f16)    # [2t, 96]: ux@0, uy@32, uz@64
        nc.gpsimd.memset(stileB[:], 0.0)
        nc.gpsimd.memset(lwall[:], 0.0)
        nc.gpsimd.memset(utile[:], 0.0)
        sB = stileB[:].rearrange("p (t c) -> p t c", t=2)
        lw4 = lwall[:].rearrange("p (t m c) -> p t m c", t=2, m=3)
        ut = utile[:].rearrange("p (t c) -> p t c", t=2)
        dsc = sbuf([96, 40], "dsc")              # compact d per slot
        nc.scalar.activation(dsc[:], lnd[:], AF.Exp, bias=0.0, scale=0.5)
        nc.vector.tensor_copy(sB[:, :, 0:20],
                              dsc[:].rearrange("p (t k) -> p t k", t=2))
        nc.vector.tensor_scalar(sB[:, :, 32:52],
                                d2s[:].rearrange("p (t k) -> p t k", t=2),
                                2.0, None, ALU.mult, ALU.bypass)

        # ---------------- gather ----------------
        # select-mask by key-value equality: mask[p,(t,k),j] = (w[p,t,j]==mvc)
        # (f32 ties are ~never; empty slots match many j -> garbage u, killed
        #  by vkill downstream, same as the index path)
        i33 = sbuf([96, 1920], "i33", f16)
        i3m = i33[:].rearrange("p (t k j) -> p t k j", t=2, j=48)
        mv_b = mvc[:].rearrange("p (t k o) -> p t k o", t=2, o=1).broadcast_to(
            [96, 2, 20, 48])
        w_b2 = w[:].rearrange("p (t o j) -> p t o j", t=2, o=1).broadcast_to(
            [96, 2, 20, 48])
        nc.vector.tensor_tensor(i3m, w_b2, mv_b, ALU.is_equal)
        # z channel first: its u-ops run on Pool, overlapping the xy path
        i3t = i33[:].rearrange("p (t k j) -> p t k j", t=2, j=48)
        gz = sbuf([96, 1920], "gz", f16)
        cz_b = data[:, _D_CZ:_D_CZ + 48].bitcast(f16).rearrange(
            "p (t o j) -> p t o j", t=2, o=1).broadcast_to([96, 2, 20, 48])
        nc.vector.tensor_tensor(
            gz[:].rearrange("p (t k j) -> p t k j", t=2, j=48), i3t, cz_b,
            ALU.mult)
        zg = sbuf([96, 40], "zg")
        nc.vector.tensor_reduce(zg[:].rearrange("p (tk o) -> p tk o", o=1),
                                gz[:].rearrange("p (tk j) -> p tk j", j=48),
                                AX.X, ALU.add)
        dxz = sbuf([96, 40], "dxz")
        dxz2 = dxz[:].rearrange("p (t k) -> p t k", t=2)
        ctrz = ctr[:, :, 2:3].broadcast_to([96, 2, 20])
        nc.gpsimd.tensor_tensor(dxz2, zg[:].rearrange("p (t k) -> p t k", t=2),
                                ctrz, ALU.subtract)
        nc.gpsimd.tensor_tensor(ut[:, :, 64:84], dxz2,
                                ivd[:].rearrange("p (t k) -> p t k", t=2),
                                ALU.mult)
        gp = sbuf([96, 1920], "gp")
        cpk_b = data[:, _D_CPK:_D_CPK + 96].rearrange(
            "p (t o j) -> p t o j", t=2, o=1).broadcast_to([96, 2, 20, 48])
        nc.vector.tensor_tensor(
            gp[:].rearrange("p (t k j) -> p t k j", t=2, j=48), i3t, cpk_b,
            ALU.mult)
        g = sbuf([96, 40], "g")
        nc.vector.tensor_reduce(g[:].rearrange("p (tk o) -> p tk o", o=1),
                                gp[:].rearrange("p (tk j) -> p tk j", j=48),
                                AX.X, ALU.add)
        # unpack packed xy via f32 round-to-int trick:
        #   A = g*2^-11 - (0.5 - 2^-12); qxp = (A + 2^23) - 2^23 = qx + 2^11
        #   qy = g - qxp*2^11
        qy = sbuf([96, 40], "qy")
        th = sbuf([96, 40], "th")
        xr = sbuf([96, 40], "xr")
        yr = sbuf([96, 40], "yr")
        _c = 0.5 - 2.0 ** -12
        nc.vector.tensor_scalar(th[:], g[:], 2.0 ** -11, -_c, ALU.mult, ALU.add)
        nc.vector.tensor_scalar(th[:], th[:], 2.0 ** 23, -(2.0 ** 23),
                                ALU.add, ALU.add)
        nc.vector.scalar_tensor_tensor(qy[:], th[:], -(2.0 ** 11), g[:],
                                       ALU.mult, ALU.add)
        # x = (qxp - 2^11) * 10/2047 = qxp*Y - 2^11*Y
        nc.vector.tensor_scalar(xr[:], th[:], _Y_UNP, -(2.0 ** 11) * _Y_UNP,
                                ALU.mult, ALU.add)
        nc.vector.tensor_scalar(yr[:], qy[:], _Y_UNP, None, ALU.mult, ALU.bypass)
        # u channels: (coord - ctr) * ivd -> utile f16 (written in place)
        for c, src in ((0, xr), (1, yr)):
            dx = sbuf([96, 40], f"dx{c}")
            dx2 = dx[:].rearrange("p (t k) -> p t k", t=2)
            ctrc = ctr[:, :, c:c + 1].broadcast_to([96, 2, 20])
            nc.vector.tensor_tensor(dx2,
                                    src[:].rearrange("p (t k) -> p t k", t=2),
                                    ctrc, ALU.subtract)
            nc.vector.tensor_tensor(ut[:, :, 32 * c:32 * c + 20], dx2,
                                    ivd[:].rearrange("p (t k) -> p t k", t=2),
                                    ALU.mult)

        # ---------------- fc_a + LW channels ----------------
        az = sbuf([96, 40], "az")
        az2 = sbuf([96, 40], "az2")
        ah = sbuf([96, 40], "ah")
        aptmp = sbuf([96, 40], "aptmp")
        # no min(ds, RCA) needed: ds = sqrt(RCA^2 - w) <= RCA by construction
        az3 = az[:].rearrange("p (o c) -> p o c", o=1)
        dsc3 = dsc[:].rearrange("p (o c) -> p o c", o=1)
        nc.gpsimd.tensor_tensor(az3, dsc3, ccol(8, 96, 40), ALU.mult)
        nc.gpsimd.tensor_tensor(az3, az3, ccol(5, 96, 40), ALU.add)
        nc.gpsimd.tensor_tensor(az2[:], az[:], az[:], ALU.mult)
        poly_sin(ah[:], az2[:], az[:], aptmp[:], 96, 40)
        kh = sbuf([96, 40], "kh")
        kh3 = kh[:].rearrange("p (o c) -> p o c", o=1)
        nc.gpsimd.tensor_tensor(kh3, ah[:].rearrange("p (o c) -> p o c", o=1),
                                ccol(5, 96, 40), ALU.mult)
        nc.gpsimd.tensor_tensor(kh3, kh3, ccol(9, 96, 40), ALU.add)
        nc.vector.tensor_scalar(kh[:], kh[:], 1e-30, None, ALU.max, ALU.bypass)
        lnfc = sbuf([96, 40], "lnfc")
        nc.scalar.activation(lnfc[:], kh[:], AF.Ln, bias=0.0, scale=1.0)
        # LW_m = -2*(ds - shfa_m)^2 + lnfc + bw_m/2  (batched over m, t outer)
        am = sbuf([96, 160], "am")
        am4 = am[:].rearrange("p (t m k) -> p t m k", t=2, m=4)
        ds_b = dsc[:].rearrange("p (t o k) -> p t o k", t=2, o=1).broadcast_to(
            [96, 2, 4, 20])
        shfarow = CF[0:96, _C_SHFA:_C_SHFA + 160].rearrange(
            "p (t m k) -> p t m k", t=2, m=4)
        nc.vector.tensor_tensor(am4, ds_b, shfarow, ALU.subtract)
        nc.vector.scalar_tensor_tensor(am[:], am[:], -2.0, am[:],
                                       ALU.mult, ALU.mult)
        lnfc_b = lnfc[:].rearrange("p (t o k) -> p t o k", t=2, o=1).broadcast_to(
            [96, 2, 4, 20])
        bwrow = CF[0:96, _C_BW:_C_BW + 160].rearrange(
            "p (t m k) -> p t m k", t=2, m=4)
        lnb = sbuf([96, 160], "lnb")
        lnb4 = lnb[:].rearrange("p (t m k) -> p t m k", t=2, m=4)
        nc.vector.tensor_tensor(lnb4, lnfc_b, bwrow, ALU.add)
        nc.vector.tensor_tensor(lw4[:, :, :, 0:20], am4[:, :, 0:3, :],
                                lnb4[:, :, 0:3, :], ALU.add)
        nc.vector.tensor_tensor(sB[:, :, 64:84], am4[:, :, 3, :],
                                lnb4[:, :, 3, :], ALU.add)

        # ---------------- hi/lo f16 split of f32 slot channels ----------------
        # v = hi + lo reconstructs f32 precision; the pair matmuls then run as
        # two accumulating f16 matmuls (1 cyc/row) instead of one f32 (4 cyc)
        lwH = sbuf([96, 192], "lwH", f16)
        lwL = sbuf([96, 192], "lwL", f16)
        sbH = sbuf([96, 192], "sbH", f16)
        sbL = sbuf([96, 192], "sbL", f16)
        nc.scalar.copy(lwH[:], lwall[:])
        nc.vector.tensor_tensor(lwL[:], lwall[:], lwH[:], ALU.subtract)
        nc.scalar.copy(sbH[:], stileB[:])
        nc.vector.tensor_tensor(sbL[:], stileB[:], sbH[:], ALU.subtract)

        # ---------------- transposes to SD ----------------
        # hi and lo transposed into one psum bank -> one copy per (src, t)
        idt16 = CF[0:96, _C_IDT16:_C_IDT16 + 48].bitcast(f16)
        SDu = sbuf([96, 192], "sdu", f16)      # ux@0, uy@32, uz@64
        SDlw = sbuf([96, 384], "sdlw", f16)    # [hl, at]: LW0@0, LW1@32, LW2@64
        SDb = sbuf([96, 384], "sdb", f16)      # [hl, at]: S@0, SQ2@32, LW3@64
        for t in range(2):
            tp = pbank(0, 96, 96, dt=f16)
            nc.tensor.transpose(tp, ut[:, t, :], idt16)
            nc.vector.tensor_copy(SDu[:, 96 * t:96 * t + 96], tp)
            for hi_, lo_, dst_, eng in ((lwH, lwL, SDlw, nc.scalar),
                                        (sbH, sbL, SDb, nc.vector)):
                tpx = pbank(0, 96, 192, dt=f16)
                nc.tensor.transpose(tpx[:, 0:96], hi_[:, 96 * t:96 * t + 96],
                                    idt16)
                nc.tensor.transpose(tpx[:, 96:192], lo_[:, 96 * t:96 * t + 96],
                                    idt16)
                dv = dst_[:].rearrange("p (hl c) -> p hl c", hl=2)[
                    :, :, 96 * t:96 * t + 96]
                sv = tpx[:, 0:192].rearrange("p (hl c) -> p hl c", hl=2)
                if eng is nc.scalar:
                    eng.copy(dv, sv)
                else:
                    eng.tensor_copy(dv, sv)

        # ---------------- pair space ----------------
        expd16 = CF[:, _C_EXPD16:_C_EXPD16 + 95].bitcast(f16)
        exps16 = CF[:, _C_EXPS16:_C_EXPS16 + 95].bitcast(f16)
        buck16 = CF[0:CW, _C_BUCK:_C_BUCK + 32].bitcast(f16)
        shh2 = sbuf([CW, 384], "shh2")
        b4 = sbuf([CW, 384], "b4")
        lwb = sbuf([CW, 1536], "lwb")
        lwb4 = lwb[:].rearrange("p (m c) -> p m c", m=4)
        for ci in range(2):
            c0 = CW * ci
            cs = slice(192 * ci, 192 * ci + 192)
            vd = [pbank(0, CW, 192) for _ in range(3)]
            for c in range(3):
                nc.tensor.matmul(vd[c], expd16[32 * c:32 * c + K, c0:c0 + CW],
                                 SDu[32 * c:32 * c + K, :], start=True,
                                 stop=True)

            def hilo_mm(bank, base, src16):
                lhsT = exps16[base:base + K, c0:c0 + CW]
                for hl in range(2):
                    nc.tensor.matmul(bank, lhsT,
                                     src16[base:base + K,
                                           192 * hl:192 * hl + 192],
                                     start=(hl == 0), stop=(hl == 1))

            pS = pbank(0, CW, 192)
            pQ = pbank(0, CW, 192)
            hilo_mm(pS, 0, SDb)
            hilo_mm(pQ, 32, SDb)
            pL = [pbank(0, CW, 192) for _ in range(4)]
            for m in range(3):
                hilo_mm(pL[m], 32 * m, SDlw)
            hilo_mm(pL[3], 64, SDb)
            # shh2 = sum_c vd_c^2
            tq = sbuf([CW, 192], f"tq{ci}")
            tq2 = sbuf([CW, 192], f"tq2{ci}")
            nc.scalar.activation(shh2[:, cs], vd[0], AF.Square, bias=0.0,
                                 scale=1.0)
            nc.scalar.activation(tq[:], vd[1], AF.Square, bias=0.0, scale=1.0)
            nc.scalar.activation(tq2[:], vd[2], AF.Square, bias=0.0, scale=1.0)
            nc.vector.tensor_tensor(shh2[:, cs], shh2[:, cs], tq[:], ALU.add)
            nc.vector.tensor_tensor(shh2[:, cs], shh2[:, cs], tq2[:], ALU.add)
            # per-chunk clip so lnc/chp don't wait for the lwb adds below
            nc.vector.tensor_scalar(shh2[:, cs], shh2[:, cs], 0.0, 4.0,
                                    ALU.max, ALU.min)
            # b4 = -2*S^2 + SQ2'
            s2 = sbuf([CW, 192], f"s2{ci}")
            nc.scalar.activation(s2[:], pS, AF.Square, bias=0.0, scale=1.0)
            nc.vector.scalar_tensor_tensor(b4[:, cs], s2[:], -2.0, pQ,
                                           ALU.mult, ALU.add)
            for m in range(4):
                nc.vector.tensor_tensor(lwb4[:, m, cs], pL[m], b4[:, cs],
                                        ALU.add)
        lnc = sbuf([CW, 384], "lnc")
        lns = sbuf([CW, 384], "lns")
        chp = sbuf([CW, 384], "chp")
        shp = sbuf([CW, 384], "shp")
        nc.scalar.activation(lnc[:], shh2[:], AF.Ln,
                             bias=CF[0:CW, _C_B975:_C_B975 + 1], scale=-0.2375)
        nc.scalar.activation(chp[:], lnc[:], AF.Exp, bias=0.0, scale=0.5)
        nc.scalar.activation(lns[:], shh2[:], AF.Ln,
                             bias=CF[0:CW, _C_B025:_C_B025 + 1], scale=0.2375)
        nc.scalar.activation(shp[:], lns[:], AF.Exp, bias=0.0, scale=0.5)
        wm = sbuf([CW, 1536], "wm", f16)
        wmv = wm[:].rearrange("p (m c) -> p m c", m=4)
        for ci in range(2):
            cc = slice(192 * ci, 192 * ci + 192)
            nc.scalar.activation(wmv[:, :, cc], lwb4[:, :, cc], AF.Exp,
                                 bias=0.0, scale=1.0)
        # tb_z = shp*tan(shfz/2) + chp  (f32), half-batched Ln, per-z biased Exp
        tb = sbuf([CW, 3072], "tb")
        tb8 = tb[:].rearrange("p (z c) -> p z c", z=8)
        for z in range(8):
            tanz = float(np.tan(SHFZ[z] / 2))
            nc.vector.scalar_tensor_tensor(tb8[:, z, :], shp[:], tanz, chp[:],
                                           ALU.mult, ALU.add)
        lnv = sbuf([CW, 3072], "lnv")
        nc.scalar.activation(lnv[:, 0:1536], tb[:, 0:1536], AF.Ln, bias=0.0,
                             scale=1.0)
        nc.scalar.activation(lnv[:, 1536:3072], tb[:, 1536:3072], AF.Ln,
                             bias=0.0, scale=1.0)
        f1 = sbuf([CW, 3072], "f1", f16)
        f18 = f1[:].rearrange("p (z c) -> p z c", z=8)
        lnv8 = lnv[:].rearrange("p (z c) -> p z c", z=8)
        # terms laid out z-major: terms[p, z, m, c] = wm[p, m, c] * f1[p, z, c]
        # flat index f = z*4 + m; bucket group g covers f in {2g, 2g+1}
        terms = sbuf([CW, 12288], "terms", f16)
        t4 = terms[:].rearrange("p (z m c) -> p z m c", z=8, m=4)
        wm4 = wm[:].rearrange("p (m c) -> p m c", m=4)
        outa_sb = sbuf([96, 2304], "outasb", f16)
        outa_banks = []
        for z in range(8):
            nc.scalar.activation(f18[:, z, :], lnv8[:, z, :], AF.Exp,
                                 bias=CF[0:CW, _C_BZ + z:_C_BZ + z + 1],
                                 scale=64.0)
            f1_b = f18[:, z, :].rearrange("p (o c) -> p o c", o=1).broadcast_to(
                [CW, 4, 384])
            nc.vector.tensor_tensor(t4[:, z, :, :], wm4, f1_b, ALU.mult)
            # two bucket groups per z (m 0-1 and m 2-3), 3 groups per psum bank
            for half in range(2):
                gidx = 2 * z + half
                bank_i, slot = divmod(gidx, 3)
                if slot == 0:
                    outa_banks.append(psum.tile(
                        [128, 512], f32, name=f"obank{bank_i}",
                        tag="obank", bufs=2))
                pA = outa_banks[bank_i][32 * slot:32 * slot + 32, 0:384]
                for ci in range(2):
                    nc.tensor.matmul(
                        pA, buck16[:, 32 * ci:32 * ci + 32],
                        t4[:, z, 2 * half:2 * half + 2, 192 * ci:192 * ci + 192],
                        start=(ci == 0), stop=(ci == 1))
                if slot == 2 or gidx == 15:
                    np_ = 32 * (slot + 1)
                    dst = outa_sb[0:np_, 384 * bank_i:384 * bank_i + 384]
                    src = outa_banks[bank_i][0:np_, 0:384]
                    if bank_i < 4:
                        nc.scalar.copy(dst, src)
                    else:
                        nc.vector.tensor_copy(dst, src)
                if gidx == 8:
                    nc.sync.dma_start(P_outa[:, 0:1152], outa_sb[:, 0:1152])
                elif gidx == 14:
                    nc.sync.dma_start(P_outa[:, 1152:1920],
                                      outa_sb[:, 1152:1920])
        nc.sync.dma_start(P_outa[0:32, 1920:2304], outa_sb[0:32, 1920:2304])

    nc.compile()
    return nc


def _pack_inputs(species, coords):
    sp = species.astype(np.int64)
    co = coords.astype(np.float32)
    cp = _BUILT["cp"]
    qx = np.round(co[..., 0] * QS)
    qy = np.round(co[..., 1] * QS)
    packed = (2.0 ** 22 + qx * 2048.0 + qy).astype(np.float32)
    z16 = co[..., 2].astype(np.float16)
    in_maps = []
    for c in range(8):
        data = np.zeros((96, _D_W), np.float32)
        oh = np.zeros((96, 2, 8), np.float16)
        for t in range(2):
            for m in range(2):
                mol = 4 * c + 2 * t + m
                rows = slice(48 * m, 48 * m + 48)
                data[rows, _D_CROW + 288 * 0:_D_CROW + 288][
                    :, 144 * t:144 * t + 144] = co[mol].reshape(-1)[None, :]
                data[rows, _D_CTR + 3 * t:_D_CTR + 3 * t + 3] = co[mol]
                data[rows, _D_CPK + 48 * t:_D_CPK + 48 * t + 48] = \
                    packed[mol][None, :]
                zv = z16[mol].view(np.float32)
                data[rows, _D_CZ + 24 * t:_D_CZ + 24 * t + 24] = zv[None, :]
                for s in range(4):
                    data[rows, _D_SPM + 192 * t + 48 * s:
                         _D_SPM + 192 * t + 48 * s + 48] = \
                        (sp[mol] == s).astype(np.float32)[None, :]
                for s in range(4):
                    oh[rows, t, 4 * m + s] = (sp[mol] == s).astype(np.float16)
        data[:, _D_OH8:_D_OH8 + 8] = oh.reshape(96, 16).view(np.float32)
        in_maps.append(dict(data=data, consts=cp))
    return in_maps


def kernel(species, coordinates):
    import sys
    sys.path.insert(0, "/opt/trn_rl_repo")
    from concourse.bass_utils import run_bass_kernel_spmd

    species = np.asarray(species)
    coords = np.asarray(coordinates, dtype=np.float32)
    N = species.shape[0]
    if "nc" not in _BUILT:
        _BUILT["cp"] = _constpack()
        _BUILT["nc"] = _build()
    nc = _BUILT["nc"]

    in_maps = _pack_inputs(species, coords)
    res = run_bass_kernel_spmd(nc, in_maps, list(range(8)))
    full = np.zeros((N, 48, 384), np.float32)
    for c in range(8):
        outr = np.asarray(res.results[c]["outr"]).astype(np.float32)
        outa = np.asarray(res.results[c]["outa"]).astype(np.float32)
        # radial: outr[p=(m,s), t*768 + f*48 + j] -> mol 4c+2t+m, atom j, s*16+f
        r = outr.reshape(2, 4, 2, 16, 48)       # [m, s, t, f, j]
        rad = r.transpose(2, 0, 4, 1, 3).reshape(2, 2, 48, 64)  # [t, m, j, sf]
        for t in range(2):
            for m in range(2):
                full[4 * c + 2 * t + m, :, 0:64] = rad[t, m]
        # angular: outa[32*slot + b, 384*i + k*192 + t*96 + mm*48 + a]
        #   g = 3i + slot; z = g//2; half = g%2; m = 2*half + k
        #   feature = 64 + b*32 + m*8 + z
        a = outa.reshape(3, 32, 6, 2, 2, 2, 48)   # [slot, 32p, i, k, t, mm, a]
        a = a[:, 0:10]                             # valid buckets
        for g in range(16):
            i, slot = divmod(g, 3)
            z, half = divmod(g, 2)
            for k in range(2):
                m = 2 * half + k
                feat = 64 + np.arange(10) * 32 + m * 8 + z
                for t in range(2):
                    for mm in range(2):
                        mol = 4 * c + 2 * t + mm
                        full[mol, :, feat] = a[slot, :, i, k, t, mm, :]
    return full
